# revision 1
# baseline (speedup 1.0000x reference)
"""GroupedQueryAttention Trainium2 kernel.

Problem shapes (hardcoded): x [2, 2048, 1024], H=16 heads, G=4 kv-groups,
head_dim=64.  out = softmax((xWq)(xWk)^T / 8) (xWv) Wo + biases.

Sharding: 8 cores, core d = (b, j) with b = d // 4, j = d % 4.
Each core computes the full attention output for batch b, query rows
[512j, 512j+512), all 16 heads — output rows are complete per core, so the
host-side gather is a pure concat (no reduction).
K/V are computed per-core for the whole batch (cheap 4x duplication).
The token axis of x^T is rolled per-core so queries are always columns
0:512 (attention is permutation-invariant over keys), keeping the SPMD
program identical across cores.

On-chip dataflow (per core), "feature-on-partition" layout, bf16 matmuls
with fp32 PSUM accumulation (softmax normalization kept in fp32r):
  x^T is pre-transposed + bf16-cast on host and DMA'd directly.
  Q^T[d,nq]  = Wq^T x_q^T   (PSUM accum over c-chunks)
  K^T[dg,n]  = Wk^T x_b^T
  V[n,dg]    = x_b Wv       (natural layout, + ones column for softmax denom)
  S^T[k,nq]  = K Q^T        (per head, per 128-k-chunk; PSUM fp32)
  P^T        = exp(S^T / 8) (ScalarE from PSUM, scale folded; no max
                             subtraction — logits are O(1) here)
  O^T[65,nq] = [V|1]^T P^T  (PSUM accum over k-chunks; row 64 = denominator)
  normalize  : reciprocal of denoms (spread over partition bases 0/32/64/96
               so DVE uses many lanes), broadcast per head via K=1 matmuls,
               one vector multiply per c-chunk
  y[nq, c]   = O^T^T Wo     (accumulate over c-chunks) + bo
"""

import ml_dtypes
import numpy as np

import concourse.bacc as bacc
import concourse.mybir as mybir
import concourse.tile as tile
from concourse.bass_utils import run_bass_kernel_spmd

# ---- problem constants (hardcoded per contract) ----
B, N, C = 2, 2048, 1024
H, G, HD = 16, 4, 64
DG = G * HD            # 256
NCORES = 8
SPLIT = NCORES // B    # 4 query splits per batch
NQ = N // SPLIT        # 512 query rows per core
P = 128
CT = C // P            # 8 c-chunks
KC = N // P            # 16 k-chunks
SB = 2                 # score k-chunks per PSUM batch (exp granularity)
SCALE = HD ** -0.5

F32 = mybir.dt.float32
F32R = mybir.dt.float32r
BF16 = mybir.dt.bfloat16
NPBF = ml_dtypes.bfloat16

_CACHE = {}


def _build():
    nc = bacc.Bacc(None, target_bir_lowering=False)

    xbT = nc.declare_dram_parameter("xbT", [C, N], F32R, isOutput=False)
    Wq = nc.declare_dram_parameter("Wq", [C, C], F32R, isOutput=False)
    Wk = nc.declare_dram_parameter("Wk", [C, DG], F32R, isOutput=False)
    Wv = nc.declare_dram_parameter("Wv", [C, DG], F32R, isOutput=False)
    Wo = nc.declare_dram_parameter("Wo", [C, C], F32R, isOutput=False)
    bq = nc.declare_dram_parameter("bq", [C], F32, isOutput=False)
    bk = nc.declare_dram_parameter("bk", [DG], F32, isOutput=False)
    bv = nc.declare_dram_parameter("bv", [DG], F32R, isOutput=False)
    bo = nc.declare_dram_parameter("bo", [C], F32R, isOutput=False)
    y = nc.declare_dram_parameter("y", [NQ, C], F32, isOutput=True)

    with tile.TileContext(nc) as tc:
        # -------- persistent tiles (live through attention) --------
        with tc.tile_pool(name="main", bufs=1) as main:
            qT = main.tile([P, CT, NQ], F32R)         # Q^T  d-chunk x q
            kT = main.tile([P, 2, N], F32R)           # K^T  dg-chunk x k
            vA = main.tile([P, KC, G, HD + 1], F32R)  # V + ones col, per k-chunk
            bqk = main.tile([P, CT + 2], F32)         # bq (d-chunked) | bk
            misc = main.tile([1, 3 * P], F32R)
            bvr = main.tile([1, DG], F32R)
            bor = main.tile([1, C], F32R)
            ones1 = misc[0:1, 0:P]
            e_lo = misc[0:1, P:P + P]
            e_hi = misc[0:1, 2 * P:3 * P]
            bqp = bqk[:, 0:CT]
            bkp = bqk[:, CT:CT + 2]

            # constants DMA'd from NEFF-embedded data (memset can't write f32r)
            cdat = np.zeros((1, 3 * P), np.float32)
            cdat[0, 0:P] = 1.0                   # ones1
            cdat[0, P:P + HD] = 1.0              # e_lo: even heads -> rows 0..63
            cdat[0, 2 * P + HD:3 * P] = 1.0      # e_hi: odd heads -> rows 64..127
            nc.sync.dma_start(out=misc[:],
                              in_=nc.inline_tensor(cdat, "consts")[:].bitcast(F32R))
            vcol_np = np.ones((P, KC * G), np.float32)
            nc.sync.dma_start(
                out=vA[:, :, :, HD:HD + 1],
                in_=nc.inline_tensor(vcol_np, "vcol")[:].bitcast(F32R)
                .rearrange("p (k g o) -> p k g o", g=G, o=1))

            nc.sync.dma_start(out=bqp, in_=bq.rearrange("(t p) -> p t", p=P))
            nc.sync.dma_start(out=bkp, in_=bk.rearrange("(t p) -> p t", p=P))
            nc.sync.dma_start(out=bvr[:], in_=bv.rearrange("(o d) -> o d", o=1))
            nc.sync.dma_start(out=bor[:], in_=bo.rearrange("(o d) -> o d", o=1))

            # ---------------- phase A+B: load + projections ----------------
            with tc.tile_pool(name="proj", bufs=1) as proj, \
                 tc.tile_pool(name="pp", bufs=2, space="PSUM") as pp:
                xbTs = proj.tile([P, CT, N], F32R)
                wq = proj.tile([P, CT, C], F32R)
                wk = proj.tile([P, CT, DG], F32R)
                wv = proj.tile([P, CT, DG], F32R)
                for t in range(CT):
                    nc.sync.dma_start(out=wq[:, t, :], in_=Wq[t * P:(t + 1) * P, :])
                    nc.sync.dma_start(out=wk[:, t, :], in_=Wk[t * P:(t + 1) * P, :])
                    nc.sync.dma_start(out=wv[:, t, :], in_=Wv[t * P:(t + 1) * P, :])
                    nc.sync.dma_start(out=xbTs[:, t, :], in_=xbT[t * P:(t + 1) * P, :])

                # Q^T [c-chunk t -> d-chunk dt]
                for dt_ in range(CT):
                    pq = pp.tile([P, NQ], F32, tag="pk")
                    for t in range(CT):
                        nc.tensor.matmul(
                            pq[:], wq[:, t, dt_ * P:(dt_ + 1) * P],
                            xbTs[:, t, 0:NQ], start=(t == 0), stop=(t == CT - 1))
                    nc.vector.tensor_scalar_add(qT[:, dt_, :], pq[:], bqp[:, dt_:dt_ + 1])

                # K^T
                for gt in range(2):
                    for nf in range(N // 512):
                        pk = pp.tile([P, 512], F32, tag="pk")
                        for t in range(CT):
                            nc.tensor.matmul(
                                pk[:], wk[:, t, gt * P:(gt + 1) * P],
                                xbTs[:, t, nf * 512:(nf + 1) * 512],
                                start=(t == 0), stop=(t == CT - 1))
                        nc.vector.tensor_scalar_add(
                            kT[:, gt, nf * 512:(nf + 1) * 512], pk[:], bkp[:, gt:gt + 1])

                # V natural + bias (+ones col preset above)
                for kc in range(KC):
                    pv = pp.tile([P, DG], F32, tag="pv")
                    for t in range(CT):
                        nc.tensor.matmul(
                            pv[:], xbTs[:, t, kc * P:(kc + 1) * P],
                            wv[:, t, :], start=(t == 0), stop=False)
                    nc.tensor.matmul(pv[:], ones1[:], bvr[:],
                                     start=False, stop=True)
                    nc.vector.tensor_copy(
                        vA[:, kc, :, 0:HD],
                        pv[:].rearrange("p (g d) -> p g d", g=G))

            # -------- phase C: attention (per head) --------
            with tc.tile_pool(name="late", bufs=1) as late:
                wo = late.tile([P, CT, C], F32R)
                oT = late.tile([P, CT, NQ], F32R)     # O^T (unnorm, then normed)
                rD = late.tile([1, H, NQ], F32R)      # per-head denom recips
                for t in range(CT):
                    nc.sync.dma_start(out=wo[:, t, :], in_=Wo[t * P:(t + 1) * P, :])

                with tc.tile_pool(name="pt", bufs=3) as ptp, \
                     tc.tile_pool(name="ps", bufs=3, space="PSUM") as psp, \
                     tc.tile_pool(name="po", bufs=2, space="PSUM") as pop:
                    for h in range(H):
                        g = h % G
                        gt, gr = g // 2, (g % 2) * HD
                        qrow = (h % 2) * HD
                        q_h = qT[qrow:qrow + HD, h // 2, :]           # [64, 512]
                        po = pop.tile([HD + 1, NQ], F32)
                        for kb in range(KC // SB):
                            ps = psp.tile([P, SB, NQ], F32)
                            for i in range(SB):
                                kc = kb * SB + i
                                nc.tensor.matmul(
                                    ps[:, i, :],
                                    kT[gr:gr + HD, gt, kc * P:(kc + 1) * P],
                                    q_h, start=True, stop=True)
                            pT = ptp.tile([P, SB, NQ], F32R)
                            nc.scalar.activation(pT[:], ps[:],
                                                 mybir.ActivationFunctionType.Exp,
                                                 scale=SCALE)
                            for i in range(SB):
                                kc = kb * SB + i
                                nc.tensor.matmul(
                                    po[:], vA[:, kc, g, :], pT[:, i, :],
                                    start=(kb == 0 and i == 0),
                                    stop=(kb == KC // SB - 1 and i == SB - 1))
                        nc.vector.tensor_copy(oT[qrow:qrow + HD, h // 2, :], po[0:HD, :])
                        nc.vector.tensor_copy(rD[0:1, h, :], po[HD:HD + 1, :])
                        with nc.allow_low_precision(reason="softmax recip f32r"):
                            nc.vector.reciprocal(rD[0:1, h, :], rD[0:1, h, :])

                # -------- normalize + out-proj --------
                with tc.tile_pool(name="pb", bufs=2, space="PSUM") as pbp, \
                     tc.tile_pool(name="ysb", bufs=2) as ysb:
                    for t in range(CT):
                        pb = pbp.tile([P, NQ], F32, tag="pb")
                        nc.tensor.matmul(pb[:], e_lo, rD[0:1, 2 * t, :],
                                         start=True, stop=False)
                        nc.tensor.matmul(pb[:], e_hi, rD[0:1, 2 * t + 1, :],
                                         start=False, stop=True)
                        nc.vector.tensor_mul(oT[:, t, :], oT[:, t, :], pb[:])

                    for m in range(NQ // P):
                        for fh in range(C // 512):
                            py = pbp.tile([P, 512], F32, tag="py")
                            for t in range(CT):
                                nc.tensor.matmul(
                                    py[:], oT[:, t, m * P:(m + 1) * P],
                                    wo[:, t, fh * 512:(fh + 1) * 512],
                                    start=(t == 0), stop=False)
                            nc.tensor.matmul(py[:], ones1[:],
                                             bor[0:1, fh * 512:(fh + 1) * 512],
                                             start=False, stop=True)
                            yt = ysb.tile([P, 512], F32)
                            nc.vector.tensor_copy(yt[:], py[:])
                            nc.sync.dma_start(
                                out=y[m * P:(m + 1) * P, fh * 512:(fh + 1) * 512],
                                in_=yt[:])

    nc.compile()
    return nc


def _get_nc():
    if "nc" not in _CACHE:
        _CACHE["nc"] = _build()
    return _CACHE["nc"]


LAST_RESULTS = None


def kernel(x, Wq, bq, Wk, bk, Wv, bv, Wo, bo, trace=False, **trace_kwargs):
    x = np.asarray(x, dtype=np.float32)
    WqB = np.ascontiguousarray(np.asarray(Wq, dtype=np.float32))
    WkB = np.ascontiguousarray(np.asarray(Wk, dtype=np.float32))
    WvB = np.ascontiguousarray(np.asarray(Wv, dtype=np.float32))
    WoB = np.ascontiguousarray(np.asarray(Wo, dtype=np.float32))
    bqF = np.ascontiguousarray(np.asarray(bq, dtype=np.float32))
    bkF = np.ascontiguousarray(np.asarray(bk, dtype=np.float32))
    bvB = np.ascontiguousarray(np.asarray(bv, dtype=np.float32))
    boB = np.ascontiguousarray(np.asarray(bo, dtype=np.float32))

    nc = _get_nc()
    in_maps = []
    for d in range(NCORES):
        b, j = d // SPLIT, d % SPLIT
        # Roll the key/token axis so this core's queries are columns 0:NQ.
        # Attention is permutation-invariant over keys, so K/V built from the
        # rolled order give identical outputs.
        xbTr = np.ascontiguousarray(np.roll(x[b].T, -j * NQ, axis=1))
        in_maps.append({
            "xbT": xbTr,
            "Wq": WqB, "Wk": WkB, "Wv": WvB, "Wo": WoB,
            "bq": bqF, "bk": bkF, "bv": bvB, "bo": boB,
        })

    res = run_bass_kernel_spmd(nc, in_maps, core_ids=list(range(NCORES)),
                               trace=trace, **trace_kwargs)
    global LAST_RESULTS
    LAST_RESULTS = res

    out = np.empty((B, N, C), dtype=np.float32)
    for d in range(NCORES):
        b, j = d // SPLIT, d % SPLIT
        out[b, j * NQ:(j + 1) * NQ, :] = res.results[d]["y"]
    return out



# revision 11
# speedup vs baseline: 1.6696x; 1.6696x over previous
"""GroupedQueryAttention Trainium2 kernel (v2).

Problem shapes (hardcoded): x [2, 2048, 1024], H=16 heads, G=4 kv-groups,
head_dim=64.  out = softmax((xWq)(xWk)^T / 8) (xWv) Wo + biases.

Sharding: 8 cores, core d = (b, j) with b = d // 4, j = d % 4.
Each core computes the full attention output for batch b, query rows
[512j, 512j+512), all 16 heads.  The token axis of x^T is rolled per-core
so queries are always columns 0:512 (attention is permutation-invariant
over keys), keeping the SPMD program identical across cores.

v2 design (vs v1 baseline, 430us):
  - all matmul operands bf16 (host-cast); fp32 PSUM accumulation
  - score matmuls row-tiled: head pairs (2p, 2p+1) occupy PE array rows
    0:64 / 64:128 concurrently (head_dim=64 contraction)
  - scores land in PSUM as bf16 so an exp window of 2048 elem/partition
    fits in 2 banks; double-buffered -> ScalarE (the only exp engine,
    ~128us floor) runs back-to-back and paces the kernel
  - AV consumes exp'd probabilities [V|1]-augmented (denominator row 64)
  - per-pair denominators copied to spread partitions, batched
    reciprocal_approx_fast, broadcast back via a tiny K=2 matmul
  - Q-projection chunks interleaved into the attention phase as PE filler
"""

import os

import ml_dtypes
import numpy as np

import concourse.bacc as bacc
import concourse.mybir as mybir
import concourse.tile as tile
from concourse.bass_utils import run_bass_kernel_spmd

# ---- problem constants (hardcoded per contract) ----
B, N, C = 2, 2048, 1024
H, G, HD = 16, 4, 64
DG = G * HD            # 256
NCORES = 8
SPLIT = NCORES // B    # 4 query splits per batch
NQ = N // SPLIT        # 512 query rows per core
P = 128
CT = C // P            # 8 c-chunks
KC = N // P            # 16 k-chunks
NPAIR = H // 2         # 8 head pairs
NW = KC // 2           # 8 windows of 2 k-chunks per pair
SCALE = HD ** -0.5

F32 = mybir.dt.float32
BF16 = mybir.dt.bfloat16
NPBF = ml_dtypes.bfloat16

_CACHE = {}


def _build():
    nc = bacc.Bacc(None, target_bir_lowering=False)

    xbT = nc.declare_dram_parameter("xbT", [C, N], BF16, isOutput=False)
    Wq = nc.declare_dram_parameter("Wq", [C, C], BF16, isOutput=False)
    Wk = nc.declare_dram_parameter("Wk", [C, DG], BF16, isOutput=False)
    Wv = nc.declare_dram_parameter("Wv", [C, DG], BF16, isOutput=False)
    Wo = nc.declare_dram_parameter("Wo", [C, C], BF16, isOutput=False)
    bq = nc.declare_dram_parameter("bq", [C], F32, isOutput=False)
    bk = nc.declare_dram_parameter("bk", [DG], F32, isOutput=False)
    bv = nc.declare_dram_parameter("bv", [DG], BF16, isOutput=False)
    bo = nc.declare_dram_parameter("bo", [C], BF16, isOutput=False)
    y = nc.declare_dram_parameter("y", [NQ, C], F32, isOutput=True)
    dbg = os.environ.get("KDBG") == "1"
    if dbg:
        qTo = nc.declare_dram_parameter("qTo", [P, CT, NQ], F32, isOutput=True)
        kTo = nc.declare_dram_parameter("kTo", [P, 2, N], F32, isOutput=True)
        vAo = nc.declare_dram_parameter("vAo", [P, KC, G, HD + 1], F32,
                                        isOutput=True)
        oTo = nc.declare_dram_parameter("oTo", [P, CT, NQ], F32, isOutput=True)
        dno = nc.declare_dram_parameter("dno", [P, 4, NQ], F32, isOutput=True)
        dnro = nc.declare_dram_parameter("dnro", [P, 4, NQ], F32, isOutput=True)

    with tile.TileContext(nc) as tc:
        with tc.tile_pool(name="main", bufs=1) as main:
            # -------- persistent tiles --------
            qT = main.tile([P, CT, NQ], BF16)          # Q^T  d-chunk x q
            kT = main.tile([P, 2, N], BF16)            # K^T  dg-chunk x k
            vA = main.tile([P, KC, G, HD + 1], BF16)   # [V | 1] per k-chunk
            oT = main.tile([P, CT, NQ], BF16)          # normalized O^T
            wo = main.tile([P, CT, C], BF16)
            dn = main.tile([P, 4, NQ], F32)            # denominators (spread)
            dnr = main.tile([P, 4, NQ], F32)           # their reciprocals
            e4 = main.tile([P, HD], F32)               # norm-broadcast lhsT
            bqk = main.tile([P, CT + 2], F32)          # bq (d-chunked) | bk
            ones1 = main.tile([1, P], BF16)
            bvr = main.tile([1, DG], BF16)
            bor = main.tile([1, C], BF16)
            bqp = bqk[:, 0:CT]
            bkp = bqk[:, CT:CT + 2]

            # constants (NEFF-embedded)
            nc.sync.dma_start(out=ones1[:],
                              in_=nc.inline_tensor(
                                  np.ones((1, P), NPBF), "ones1")[:])
            vcol = np.ones((P, KC * G), NPBF)
            nc.sync.dma_start(
                out=vA[:, :, :, HD:HD + 1],
                in_=nc.inline_tensor(vcol, "vcol")[:]
                .rearrange("p (k g o) -> p k g o", g=G, o=1))
            # e4: ones rows at partitions 0/32/64/96 (K=1 broadcast lhsT)
            em = np.zeros((P, HD), np.float32)
            em[0::32, :] = 1.0
            nc.sync.dma_start(out=e4[:], in_=nc.inline_tensor(em, "e4")[:])

            nc.sync.dma_start(out=bqp, in_=bq.rearrange("(t p) -> p t", p=P))
            nc.sync.dma_start(out=bkp, in_=bk.rearrange("(t p) -> p t", p=P))
            nc.sync.dma_start(out=bvr[:], in_=bv.rearrange("(o d) -> o d", o=1))
            nc.sync.dma_start(out=bor[:], in_=bo.rearrange("(o d) -> o d", o=1))

            with tc.tile_pool(name="proj", bufs=1) as proj:
                xbTs = proj.tile([P, CT, N], BF16)
                wq = proj.tile([P, CT, C], BF16)
                wk = proj.tile([P, CT, DG], BF16)
                wv = proj.tile([P, CT, DG], BF16)

                # input DMAs, in consumption order
                nc.sync.dma_start(
                    out=wk[:], in_=Wk.rearrange("(t p) d -> p t d", p=P))
                for t in range(CT):
                    nc.sync.dma_start(out=xbTs[:, t, :],
                                      in_=xbT[t * P:(t + 1) * P, :])
                nc.sync.dma_start(
                    out=wv[:], in_=Wv.rearrange("(t p) d -> p t d", p=P))
                nc.sync.dma_start(
                    out=wq[:], in_=Wq.rearrange("(t p) d -> p t d", p=P))
                nc.sync.dma_start(
                    out=wo[:], in_=Wo.rearrange("(t p) d -> p t d", p=P))

                with tc.tile_pool(name="pp", bufs=2, space="PSUM") as pp:
                    # ---- K^T ----
                    for gt in range(2):
                        for nf in range(4):
                            pk = pp.tile([P, 512], F32, tag="pk")
                            for t in range(CT):
                                nc.tensor.matmul(
                                    pk[:], wk[:, t, gt * P:(gt + 1) * P],
                                    xbTs[:, t, nf * 512:(nf + 1) * 512],
                                    start=(t == 0), stop=(t == CT - 1))
                            nc.vector.tensor_scalar_add(
                                kT[:, gt, nf * 512:(nf + 1) * 512], pk[:],
                                bkp[:, gt:gt + 1])

                    # ---- V (natural layout) ----
                    for kc in range(KC):
                        pv = pp.tile([P, DG], F32, tag="pv")
                        for t in range(CT):
                            nc.tensor.matmul(
                                pv[:], xbTs[:, t, kc * P:(kc + 1) * P],
                                wv[:, t, :], start=(t == 0), stop=False)
                        nc.tensor.matmul(pv[:], ones1[:], bvr[:],
                                         start=False, stop=True)
                        nc.vector.tensor_copy(
                            vA[:, kc, :, 0:HD],
                            pv[:].rearrange("p (g d) -> p g d", g=G))

                    # ---- Q^T chunks 0..1 (rest interleaved into attention)
                    for dt_ in range(2):
                        pq = pp.tile([P, NQ], F32, tag="pk")
                        for t in range(CT):
                            nc.tensor.matmul(
                                pq[:], wq[:, t, dt_ * P:(dt_ + 1) * P],
                                xbTs[:, t, 0:NQ],
                                start=(t == 0), stop=(t == CT - 1))
                        nc.vector.tensor_scalar_add(
                            qT[:, dt_, :], pq[:], bqp[:, dt_:dt_ + 1])

                # -------- attention: 8 head pairs --------
                # PSUM banks: ps 2x2 + po 2x1 + qf 1 + pb 1 = 8
                with tc.tile_pool(name="ps", bufs=2, space="PSUM") as psp, \
                     tc.tile_pool(name="pt", bufs=3) as ptp, \
                     tc.tile_pool(name="po", bufs=1, space="PSUM") as pop, \
                     tc.tile_pool(name="qf", bufs=1, space="PSUM") as qfp, \
                     tc.tile_pool(name="pn", bufs=1, space="PSUM") as pnp:
                    for p_ in range(NPAIR):
                        g0, g1 = (2 * p_) % G, (2 * p_ + 1) % G
                        po0 = pop.tile([HD + 1, NQ], F32, tag="po0")
                        po1 = pop.tile([HD + 1, NQ], F32, tag="po1")
                        pos = (po0, po1)
                        dt_ = 2 + p_  # Q chunk computed as PE filler
                        pq_f = None
                        if dt_ < CT:
                            pq_f = qfp.tile([P, NQ], F32, tag="qf",
                                            name=f"pqf{p_}")
                        for kc in range(KC):
                            ps = psp.tile([P, 2, NQ], F32, tag="ps")
                            for hh in range(2):
                                r = hh * HD
                                g = g0 if hh == 0 else g1
                                nc.tensor.matmul(
                                    ps[:, hh, :],
                                    kT[r:r + HD, g // 2, kc * P:(kc + 1) * P],
                                    qT[r:r + HD, p_, :],
                                    start=True, stop=True)
                            pT = ptp.tile([P, 2, NQ], BF16, tag="pt")
                            nc.scalar.activation(
                                pT[:], ps[:],
                                mybir.ActivationFunctionType.Exp,
                                scale=SCALE)
                            for hh in range(2):
                                g = g0 if hh == 0 else g1
                                nc.tensor.matmul(
                                    pos[hh][:], vA[:, kc, g, :],
                                    pT[:, hh, :],
                                    start=(kc == 0), stop=(kc == KC - 1))
                            # PE filler: Q chunk spread over the pair
                            if pq_f is not None and kc % 2 == 0:
                                t = kc // 2
                                nc.tensor.matmul(
                                    pq_f[:], wq[:, t, dt_ * P:(dt_ + 1) * P],
                                    xbTs[:, t, 0:NQ],
                                    start=(t == 0), stop=(t == CT - 1))
                                if t == CT - 1:
                                    nc.vector.tensor_scalar_add(
                                        qT[:, dt_, :], pq_f[:],
                                        bqp[:, dt_:dt_ + 1])

                        # ---- stash pair p_ results, release po ----
                        pi = 32 * ((2 * p_) % 4)   # 0 or 64
                        slot = p_ // 2
                        nc.vector.tensor_copy(
                            dn[pi:pi + 1, slot, :], po0[HD:HD + 1, :])
                        nc.vector.tensor_copy(
                            dn[pi + 32:pi + 33, slot, :], po1[HD:HD + 1, :])
                        nc.vector.tensor_copy(oT[0:HD, p_, :], po0[0:HD, :])
                        nc.vector.tensor_copy(oT[HD:P, p_, :], po1[0:HD, :])
                        if p_ % 2 == 1:
                            # both pairs of this slot done: batched recip
                            # (full 128 partitions: base-64 slices are
                            # silently broken in the custom DVE op)
                            with nc.allow_low_precision(reason="recip"):
                                nc.vector.reciprocal_approx_fast(
                                    dnr[:, slot, :], dn[:, slot, :])
                            for pp_ in (p_ - 1, p_):
                                pj = 32 * ((2 * pp_) % 4)
                                pb = pnp.tile([P, NQ], F32, tag="pb",
                                              name=f"pb{pp_}")
                                nc.tensor.matmul(
                                    pb[0:HD, :], e4[pj:pj + 1, :],
                                    dnr[pj:pj + 1, slot, :],
                                    start=True, stop=True,
                                    tile_position=(pj, 0))
                                nc.tensor.matmul(
                                    pb[HD:P, :], e4[pj + 32:pj + 33, :],
                                    dnr[pj + 32:pj + 33, slot, :],
                                    start=True, stop=True,
                                    tile_position=(pj + 32, 64))
                                with nc.allow_low_precision(reason="bf16"):
                                    nc.vector.tensor_mul(
                                        oT[0:HD, pp_, :], oT[0:HD, pp_, :],
                                        pb[0:HD, :])
                                    nc.vector.tensor_mul(
                                        oT[HD:P, pp_, :], oT[HD:P, pp_, :],
                                        pb[HD:P, :])

            if dbg:
                with tc.tile_pool(name="dbg", bufs=1) as dbgp:
                    for (dst, srcT) in ((qTo, qT), (kTo, kT), (oTo, oT)):
                        f = dbgp.tile([P, 8 * NQ], F32, name=f"d{dst.name}")
                        nc.vector.tensor_copy(
                            f[:].rearrange("p (a b) -> p a b", a=srcT.shape[1]),
                            srcT[:])
                        nc.sync.dma_start(
                            out=dst.rearrange("p a b -> p (a b)"), in_=f[:])
                    fv = dbgp.tile([P, KC * G * (HD + 1)], F32, name="dv")
                    nc.vector.tensor_copy(
                        fv[:].rearrange("p (k g d) -> p k g d", k=KC, g=G),
                        vA[:])
                    nc.sync.dma_start(
                        out=vAo.rearrange("p k g d -> p (k g d)"), in_=fv[:])
                    nc.sync.dma_start(
                        out=dno.rearrange("p a b -> p (a b)"),
                        in_=dn[:].rearrange("p a b -> p (a b)"))
                    nc.sync.dma_start(
                        out=dnro.rearrange("p a b -> p (a b)"),
                        in_=dnr[:].rearrange("p a b -> p (a b)"))

            # -------- out-projection --------
            with tc.tile_pool(name="py", bufs=2, space="PSUM") as pyp, \
                 tc.tile_pool(name="ysb", bufs=2) as ysb:
                for m in range(NQ // P):
                    for fh in range(C // 512):
                        py = pyp.tile([P, 512], F32, tag="py")
                        for t in range(CT):
                            nc.tensor.matmul(
                                py[:], oT[:, t, m * P:(m + 1) * P],
                                wo[:, t, fh * 512:(fh + 1) * 512],
                                start=(t == 0), stop=False)
                        nc.tensor.matmul(py[:], ones1[:],
                                         bor[0:1, fh * 512:(fh + 1) * 512],
                                         start=False, stop=True)
                        yt = ysb.tile([P, 512], F32, tag="yt")
                        nc.vector.tensor_copy(yt[:], py[:])
                        nc.sync.dma_start(
                            out=y[m * P:(m + 1) * P, fh * 512:(fh + 1) * 512],
                            in_=yt[:])

    nc.compile()
    return nc


def _get_nc():
    if "nc" not in _CACHE:
        _CACHE["nc"] = _build()
    return _CACHE["nc"]


LAST_RESULTS = None


def kernel(x, Wq, bq, Wk, bk, Wv, bv, Wo, bo, trace=False, **trace_kwargs):
    x = np.asarray(x, dtype=np.float32)
    WqB = np.ascontiguousarray(np.asarray(Wq, np.float32)).astype(NPBF)
    WkB = np.ascontiguousarray(np.asarray(Wk, np.float32)).astype(NPBF)
    WvB = np.ascontiguousarray(np.asarray(Wv, np.float32)).astype(NPBF)
    WoB = np.ascontiguousarray(np.asarray(Wo, np.float32)).astype(NPBF)
    bqF = np.ascontiguousarray(np.asarray(bq, np.float32))
    bkF = np.ascontiguousarray(np.asarray(bk, np.float32))
    bvB = np.ascontiguousarray(np.asarray(bv, np.float32)).astype(NPBF)
    boB = np.ascontiguousarray(np.asarray(bo, np.float32)).astype(NPBF)

    nc = _get_nc()
    in_maps = []
    for d in range(NCORES):
        b, j = d // SPLIT, d % SPLIT
        # Roll the key/token axis so this core's queries are columns 0:NQ.
        xbTr = np.ascontiguousarray(
            np.roll(x[b].T, -j * NQ, axis=1)).astype(NPBF)
        in_maps.append({
            "xbT": xbTr,
            "Wq": WqB, "Wk": WkB, "Wv": WvB, "Wo": WoB,
            "bq": bqF, "bk": bkF, "bv": bvB, "bo": boB,
        })

    res = run_bass_kernel_spmd(nc, in_maps, core_ids=list(range(NCORES)),
                               trace=trace, **trace_kwargs)
    global LAST_RESULTS
    LAST_RESULTS = res

    out = np.empty((B, N, C), dtype=np.float32)
    for d in range(NCORES):
        b, j = d // SPLIT, d % SPLIT
        out[b, j * NQ:(j + 1) * NQ, :] = res.results[d]["y"]
    return out


# revision 15
# speedup vs baseline: 1.7557x; 1.0516x over previous
"""GroupedQueryAttention Trainium2 kernel (v2).

Problem shapes (hardcoded): x [2, 2048, 1024], H=16 heads, G=4 kv-groups,
head_dim=64.  out = softmax((xWq)(xWk)^T / 8) (xWv) Wo + biases.

Sharding: 8 cores, core d = (b, j) with b = d // 4, j = d % 4.
Each core computes the full attention output for batch b, query rows
[512j, 512j+512), all 16 heads.  The token axis of x^T is rolled per-core
so queries are always columns 0:512 (attention is permutation-invariant
over keys), keeping the SPMD program identical across cores.

v2 design (vs v1 baseline, 430us):
  - all matmul operands bf16 (host-cast); fp32 PSUM accumulation
  - score matmuls row-tiled: head pairs (2p, 2p+1) occupy PE array rows
    0:64 / 64:128 concurrently (head_dim=64 contraction)
  - scores land in PSUM as bf16 so an exp window of 2048 elem/partition
    fits in 2 banks; double-buffered -> ScalarE (the only exp engine,
    ~128us floor) runs back-to-back and paces the kernel
  - AV consumes exp'd probabilities [V|1]-augmented (denominator row 64)
  - per-pair denominators copied to spread partitions, batched
    reciprocal_approx_fast, broadcast back via a tiny K=2 matmul
  - Q-projection chunks interleaved into the attention phase as PE filler
"""

import os

import ml_dtypes
import numpy as np

import concourse.bacc as bacc
import concourse.mybir as mybir
import concourse.tile as tile
from concourse.bass_utils import run_bass_kernel_spmd

# ---- problem constants (hardcoded per contract) ----
B, N, C = 2, 2048, 1024
H, G, HD = 16, 4, 64
DG = G * HD            # 256
NCORES = 8
SPLIT = NCORES // B    # 4 query splits per batch
NQ = N // SPLIT        # 512 query rows per core
P = 128
CT = C // P            # 8 c-chunks
KC = N // P            # 16 k-chunks
NPAIR = H // 2         # 8 head pairs
NW = KC // 2           # 8 windows of 2 k-chunks per pair
SCALE = HD ** -0.5

F32 = mybir.dt.float32
BF16 = mybir.dt.bfloat16
NPBF = ml_dtypes.bfloat16

_CACHE = {}


def _build():
    nc = bacc.Bacc(None, target_bir_lowering=False)

    xbT = nc.declare_dram_parameter("xbT", [C, N], BF16, isOutput=False)
    # weights host-pre-shuffled to [p, t, d] layout for contiguous DMA
    Wq = nc.declare_dram_parameter("Wq", [P, CT * C], BF16, isOutput=False)
    Wk = nc.declare_dram_parameter("Wk", [P, CT * DG], BF16, isOutput=False)
    Wv = nc.declare_dram_parameter("Wv", [P, CT * DG], BF16, isOutput=False)
    Wo = nc.declare_dram_parameter("Wo", [P, CT * C], BF16, isOutput=False)
    bqs = nc.declare_dram_parameter("bqs", [P, CT], F32, isOutput=False)
    bks = nc.declare_dram_parameter("bks", [P, 2], F32, isOutput=False)
    bv = nc.declare_dram_parameter("bv", [DG], BF16, isOutput=False)
    bo = nc.declare_dram_parameter("bo", [C], BF16, isOutput=False)
    y = nc.declare_dram_parameter("y", [NQ, C], F32, isOutput=True)
    dbg = os.environ.get("KDBG") == "1"
    if dbg:
        qTo = nc.declare_dram_parameter("qTo", [P, CT, NQ], F32, isOutput=True)
        kTo = nc.declare_dram_parameter("kTo", [P, 2, N], F32, isOutput=True)
        vAo = nc.declare_dram_parameter("vAo", [P, KC, G, HD + 1], F32,
                                        isOutput=True)
        oTo = nc.declare_dram_parameter("oTo", [P, CT, NQ], F32, isOutput=True)
        dno = nc.declare_dram_parameter("dno", [P, 4, NQ], F32, isOutput=True)
        dnro = nc.declare_dram_parameter("dnro", [P, 4, NQ], F32, isOutput=True)

    with tile.TileContext(nc) as tc:
        with tc.tile_pool(name="main", bufs=1) as main:
            # -------- persistent tiles --------
            qT = main.tile([P, CT, NQ], BF16)          # Q^T  d-chunk x q
            kT = main.tile([P, 2, N], BF16)            # K^T  dg-chunk x k
            vA = main.tile([P, KC, G, HD + 1], BF16)   # [V | 1] per k-chunk
            oT = main.tile([P, CT, NQ], BF16)          # normalized O^T
            wo = main.tile([P, CT, C], BF16)
            dn = main.tile([P, 4, NQ], F32)            # denominators (spread)
            dnr = main.tile([P, 4, NQ], F32)           # their reciprocals
            e4 = main.tile([P, HD], BF16)
            dnb = main.tile([P, 4, NQ], BF16)               # norm-broadcast lhsT
            ypart = main.tile([P, CT, 512], F32)       # O-proj partials
            bqk = main.tile([P, CT + 2], F32)          # bq (d-chunked) | bk
            ones1 = main.tile([1, P], BF16)
            bvr = main.tile([1, DG], BF16)
            bor = main.tile([1, C], BF16)
            bqp = bqk[:, 0:CT]
            bkp = bqk[:, CT:CT + 2]

            with tc.tile_pool(name="proj", bufs=1) as proj:
                xbTs = proj.tile([P, CT, N], BF16)
                wq = proj.tile([P, CT, C], BF16)
                wk = proj.tile([P, CT, DG], BF16)
                wv = proj.tile([P, CT, DG], BF16)

                # input DMAs, in consumption order (contiguous layouts)
                nc.sync.dma_start(
                    out=wk[:], in_=Wk.rearrange("p (t d) -> p t d", t=CT))
                for t in range(CT):
                    nc.sync.dma_start(out=xbTs[:, t, :],
                                      in_=xbT[t * P:(t + 1) * P, :])
                nc.sync.dma_start(
                    out=wv[:], in_=Wv.rearrange("p (t d) -> p t d", t=CT))
                # biases (contiguous, host-pre-shuffled)
                nc.sync.dma_start(out=bqp, in_=bqs[:, :])
                nc.sync.dma_start(out=bkp, in_=bks[:, :])
                nc.sync.dma_start(out=bvr[:],
                                  in_=bv.rearrange("(o d) -> o d", o=1))
                nc.sync.dma_start(out=bor[:],
                                  in_=bo.rearrange("(o d) -> o d", o=1))
                nc.sync.dma_start(out=ones1[:],
                                  in_=nc.inline_tensor(
                                      np.ones((1, P), NPBF), "ones1")[:])
                # e4: ones rows at partitions 0/32/64/96 (K=1 bcast lhsT)
                em = np.zeros((P, HD), NPBF)
                em[0::32, :] = 1.0
                nc.sync.dma_start(out=e4[:],
                                  in_=nc.inline_tensor(em, "e4")[:])
                # vA ones column via memset (GpSimd; V copies overwrite 0:HD)
                nc.gpsimd.memset(vA[:], 1.0)
                nc.sync.dma_start(
                    out=wq[:], in_=Wq.rearrange("p (t d) -> p t d", t=CT))
                nc.sync.dma_start(
                    out=wo[:], in_=Wo.rearrange("p (t d) -> p t d", t=CT))

                with tc.tile_pool(name="pp", bufs=2, space="PSUM") as pp:
                    # ---- K^T ----
                    for gt in range(2):
                        for nf in range(4):
                            pk = pp.tile([P, 512], F32, tag="pk")
                            for t in range(CT):
                                nc.tensor.matmul(
                                    pk[:], wk[:, t, gt * P:(gt + 1) * P],
                                    xbTs[:, t, nf * 512:(nf + 1) * 512],
                                    start=(t == 0), stop=(t == CT - 1))
                            nc.vector.tensor_scalar_add(
                                kT[:, gt, nf * 512:(nf + 1) * 512], pk[:],
                                bkp[:, gt:gt + 1])

                    # ---- V (natural layout) ----
                    for kc in range(KC):
                        pv = pp.tile([P, DG], F32, tag="pv")
                        for t in range(CT):
                            nc.tensor.matmul(
                                pv[:], xbTs[:, t, kc * P:(kc + 1) * P],
                                wv[:, t, :], start=(t == 0), stop=False)
                        nc.tensor.matmul(pv[:], ones1[:], bvr[:],
                                         start=False, stop=True)
                        nc.vector.tensor_copy(
                            vA[:, kc, :, 0:HD],
                            pv[:].rearrange("p (g d) -> p g d", g=G))

                    # ---- Q^T chunks 0..1 (rest interleaved into attention)
                    for dt_ in range(2):
                        pq = pp.tile([P, NQ], F32, tag="pk")
                        for t in range(CT):
                            nc.tensor.matmul(
                                pq[:], wq[:, t, dt_ * P:(dt_ + 1) * P],
                                xbTs[:, t, 0:NQ],
                                start=(t == 0), stop=(t == CT - 1))
                        nc.vector.tensor_scalar_add(
                            qT[:, dt_, :], pq[:], bqp[:, dt_:dt_ + 1])

                # -------- attention: 8 head pairs --------
                # PSUM banks: ps 2x2 + po 2x1 + qf 1 + pb 1 = 8
                with tc.tile_pool(name="ps", bufs=2, space="PSUM") as psp, \
                     tc.tile_pool(name="pt", bufs=3) as ptp, \
                     tc.tile_pool(name="po", bufs=1, space="PSUM") as pop, \
                     tc.tile_pool(name="qf", bufs=1, space="PSUM") as qfp, \
                     tc.tile_pool(name="pn", bufs=1, space="PSUM") as pnp:
                    opart_sched = {}
                    _oi = 0
                    for _pp in (6, 7):
                        for _w in (1, 5, 9, 13):
                            opart_sched[(_pp, _w)] = _oi
                            _oi += 1
                    for p_ in range(NPAIR):
                        g0, g1 = (2 * p_) % G, (2 * p_ + 1) % G
                        po0 = pop.tile([HD + 1, NQ], F32, tag="po0")
                        po1 = pop.tile([HD + 1, NQ], F32, tag="po1")
                        pos = (po0, po1)
                        dt_ = 2 + p_  # Q chunk computed as PE filler
                        pq_f = None
                        if dt_ < CT:
                            pq_f = qfp.tile([P, NQ], F32, tag="qf",
                                            name=f"pqf{p_}")
                        for kc in range(KC):
                            ps = psp.tile([P, 2, NQ], F32, tag="ps")
                            for hh in range(2):
                                r = hh * HD
                                g = g0 if hh == 0 else g1
                                nc.tensor.matmul(
                                    ps[:, hh, :],
                                    kT[r:r + HD, g // 2, kc * P:(kc + 1) * P],
                                    qT[r:r + HD, p_, :],
                                    start=True, stop=True)
                            pT = ptp.tile([P, 2, NQ], BF16, tag="pt")
                            nc.scalar.activation(
                                pT[:], ps[:],
                                mybir.ActivationFunctionType.Exp,
                                scale=SCALE)
                            for hh in range(2):
                                g = g0 if hh == 0 else g1
                                nc.tensor.matmul(
                                    pos[hh][:], vA[:, kc, g, :],
                                    pT[:, hh, :],
                                    start=(kc == 0), stop=(kc == KC - 1))
                            # PE filler: Q chunk spread over the pair
                            if pq_f is not None and kc % 2 == 0:
                                t = kc // 2
                                nc.tensor.matmul(
                                    pq_f[:], wq[:, t, dt_ * P:(dt_ + 1) * P],
                                    xbTs[:, t, 0:NQ],
                                    start=(t == 0), stop=(t == CT - 1))
                                if t == CT - 1:
                                    nc.vector.tensor_scalar_add(
                                        qT[:, dt_, :], pq_f[:],
                                        bqp[:, dt_:dt_ + 1])
                            # PE filler: O-proj partials (chunks 0..5)
                            oi = opart_sched.get((p_, kc))
                            if oi is not None:
                                m_, fh_ = oi // 2, oi % 2
                                nt = 6 if p_ == 6 else 7
                                pyp_ = qfp.tile([P, 512], F32, tag="qf",
                                                name=f"op{oi}")
                                for t in range(nt):
                                    nc.tensor.matmul(
                                        pyp_[:], oT[:, t, m_ * P:(m_ + 1) * P],
                                        wo[:, t, fh_ * 512:(fh_ + 1) * 512],
                                        start=(t == 0), stop=(t == nt - 1))
                                nc.vector.tensor_copy(ypart[:, oi, :], pyp_[:])

                        # ---- normalize pair p_ (per-pair, keeps PE warm) --
                        pi = 32 * ((2 * p_) % 4)   # 0 or 64
                        slot = p_ // 2
                        nc.vector.tensor_copy(
                            dn[pi:pi + 1, slot, :], po0[HD:HD + 1, :])
                        nc.vector.tensor_copy(
                            dn[pi + 32:pi + 33, slot, :], po1[HD:HD + 1, :])
                        nc.vector.tensor_copy(oT[0:HD, p_, :], po0[0:HD, :])
                        nc.vector.tensor_copy(oT[HD:P, p_, :], po1[0:HD, :])
                        # full-128-partition recip (base-64 slices are broken
                        # in the custom DVE op; re-reading the sibling pair's
                        # rows is idempotent, garbage rows are never read)
                        with nc.allow_low_precision(reason="recip"):
                            nc.vector.reciprocal_approx_fast(
                                dnr[:, slot, :], dn[:, slot, :])
                            nc.vector.tensor_copy(dnb[:, slot, :],
                                                  dnr[:, slot, :])
                        pb = pnp.tile([P, NQ], F32, tag="pb",
                                      name=f"pb{p_}")
                        nc.tensor.matmul(
                            pb[0:HD, :], e4[pi:pi + 1, :],
                            dnb[pi:pi + 1, slot, :],
                            start=True, stop=True,
                            tile_position=(pi, 0))
                        nc.tensor.matmul(
                            pb[HD:P, :], e4[pi + 32:pi + 33, :],
                            dnb[pi + 32:pi + 33, slot, :],
                            start=True, stop=True,
                            tile_position=(pi + 32, 64))
                        with nc.allow_low_precision(reason="bf16"):
                            nc.vector.tensor_mul(
                                oT[0:HD, p_, :], oT[0:HD, p_, :],
                                pb[0:HD, :])
                            nc.vector.tensor_mul(
                                oT[HD:P, p_, :], oT[HD:P, p_, :],
                                pb[HD:P, :])

            if dbg:
                with tc.tile_pool(name="dbg", bufs=1) as dbgp:
                    for (dst, srcT) in ((qTo, qT), (kTo, kT), (oTo, oT)):
                        f = dbgp.tile([P, 8 * NQ], F32, name=f"d{dst.name}")
                        nc.vector.tensor_copy(
                            f[:].rearrange("p (a b) -> p a b", a=srcT.shape[1]),
                            srcT[:])
                        nc.sync.dma_start(
                            out=dst.rearrange("p a b -> p (a b)"), in_=f[:])
                    fv = dbgp.tile([P, KC * G * (HD + 1)], F32, name="dv")
                    nc.vector.tensor_copy(
                        fv[:].rearrange("p (k g d) -> p k g d", k=KC, g=G),
                        vA[:])
                    nc.sync.dma_start(
                        out=vAo.rearrange("p k g d -> p (k g d)"), in_=fv[:])
                    nc.sync.dma_start(
                        out=dno.rearrange("p a b -> p (a b)"),
                        in_=dn[:].rearrange("p a b -> p (a b)"))
                    nc.sync.dma_start(
                        out=dnro.rearrange("p a b -> p (a b)"),
                        in_=dnr[:].rearrange("p a b -> p (a b)"))

            # -------- out-projection tail (chunks 6,7 + bias + partial) ---
            with tc.tile_pool(name="py", bufs=2, space="PSUM") as pyp, \
                 tc.tile_pool(name="ysb", bufs=2) as ysb:
                for m in range(NQ // P):
                    for fh in range(C // 512):
                        oi = m * 2 + fh
                        py = pyp.tile([P, 512], F32, tag="py")
                        ts_ = (6, 7) if oi < 4 else (7,)
                        for t in ts_:
                            nc.tensor.matmul(
                                py[:], oT[:, t, m * P:(m + 1) * P],
                                wo[:, t, fh * 512:(fh + 1) * 512],
                                start=(t == ts_[0]), stop=False)
                        nc.tensor.matmul(py[:], ones1[:],
                                         bor[0:1, fh * 512:(fh + 1) * 512],
                                         start=False, stop=True)
                        yt = ysb.tile([P, 512], F32, tag="yt")
                        nc.vector.tensor_add(yt[:], py[:], ypart[:, oi, :])
                        nc.sync.dma_start(
                            out=y[m * P:(m + 1) * P, fh * 512:(fh + 1) * 512],
                            in_=yt[:])

    nc.compile()
    return nc


def _get_nc():
    if "nc" not in _CACHE:
        _CACHE["nc"] = _build()
    return _CACHE["nc"]


LAST_RESULTS = None


def _shuf(w):
    # [C, D] -> [P, CT*D]: slot (p; t, d) = w[t*128 + p, d]
    w = np.asarray(w, np.float32).astype(NPBF)
    d = w.shape[1]
    return np.ascontiguousarray(
        w.reshape(CT, P, d).transpose(1, 0, 2).reshape(P, CT * d))


def kernel(x, Wq, bq, Wk, bk, Wv, bv, Wo, bo, trace=False, **trace_kwargs):
    x = np.asarray(x, dtype=np.float32)
    WqB = _shuf(Wq)
    WkB = _shuf(Wk)
    WvB = _shuf(Wv)
    WoB = _shuf(Wo)
    bqF = np.ascontiguousarray(
        np.asarray(bq, np.float32).reshape(CT, P).T)
    bkF = np.ascontiguousarray(
        np.asarray(bk, np.float32).reshape(2, P).T)
    bvB = np.ascontiguousarray(np.asarray(bv, np.float32)).astype(NPBF)
    boB = np.ascontiguousarray(np.asarray(bo, np.float32)).astype(NPBF)

    nc = _get_nc()
    in_maps = []
    for d in range(NCORES):
        b, j = d // SPLIT, d % SPLIT
        # Roll the key/token axis so this core's queries are columns 0:NQ.
        xbTr = np.ascontiguousarray(
            np.roll(x[b].T, -j * NQ, axis=1)).astype(NPBF)
        in_maps.append({
            "xbT": xbTr,
            "Wq": WqB, "Wk": WkB, "Wv": WvB, "Wo": WoB,
            "bqs": bqF, "bks": bkF, "bv": bvB, "bo": boB,
        })

    res = run_bass_kernel_spmd(nc, in_maps, core_ids=list(range(NCORES)),
                               trace=trace, **trace_kwargs)
    global LAST_RESULTS
    LAST_RESULTS = res

    out = np.empty((B, N, C), dtype=np.float32)
    for d in range(NCORES):
        b, j = d // SPLIT, d % SPLIT
        out[b, j * NQ:(j + 1) * NQ, :] = res.results[d]["y"]
    return out


# revision 17
# speedup vs baseline: 1.7707x; 1.0085x over previous
"""GroupedQueryAttention Trainium2 kernel (v2).

Problem shapes (hardcoded): x [2, 2048, 1024], H=16 heads, G=4 kv-groups,
head_dim=64.  out = softmax((xWq)(xWk)^T / 8) (xWv) Wo + biases.

Sharding: 8 cores, core d = (b, j) with b = d // 4, j = d % 4.
Each core computes the full attention output for batch b, query rows
[512j, 512j+512), all 16 heads.  The token axis of x^T is rolled per-core
so queries are always columns 0:512 (attention is permutation-invariant
over keys), keeping the SPMD program identical across cores.

v2 design (vs v1 baseline, 430us):
  - all matmul operands bf16 (host-cast); fp32 PSUM accumulation
  - score matmuls row-tiled: head pairs (2p, 2p+1) occupy PE array rows
    0:64 / 64:128 concurrently (head_dim=64 contraction)
  - scores land in PSUM as bf16 so an exp window of 2048 elem/partition
    fits in 2 banks; double-buffered -> ScalarE (the only exp engine,
    ~128us floor) runs back-to-back and paces the kernel
  - AV consumes exp'd probabilities [V|1]-augmented (denominator row 64)
  - per-pair denominators copied to spread partitions, batched
    reciprocal_approx_fast, broadcast back via a tiny K=2 matmul
  - Q-projection chunks interleaved into the attention phase as PE filler
"""

import os

import ml_dtypes
import numpy as np

import concourse.bacc as bacc
import concourse.mybir as mybir
import concourse.tile as tile
from concourse.bass_utils import run_bass_kernel_spmd

# ---- problem constants (hardcoded per contract) ----
B, N, C = 2, 2048, 1024
H, G, HD = 16, 4, 64
DG = G * HD            # 256
NCORES = 8
SPLIT = NCORES // B    # 4 query splits per batch
NQ = N // SPLIT        # 512 query rows per core
P = 128
CT = C // P            # 8 c-chunks
KC = N // P            # 16 k-chunks
NPAIR = H // 2         # 8 head pairs
NW = KC // 2           # 8 windows of 2 k-chunks per pair
SCALE = HD ** -0.5

F32 = mybir.dt.float32
BF16 = mybir.dt.bfloat16
NPBF = ml_dtypes.bfloat16

_CACHE = {}


def _build():
    nc = bacc.Bacc(None, target_bir_lowering=False)

    xbT = nc.declare_dram_parameter("xbT", [C, N], BF16, isOutput=False)
    # weights host-pre-shuffled to [p, t, d] layout for contiguous DMA
    Wq = nc.declare_dram_parameter("Wq", [P, CT * C], BF16, isOutput=False)
    Wk = nc.declare_dram_parameter("Wk", [P, CT * DG], BF16, isOutput=False)
    Wv = nc.declare_dram_parameter("Wv", [P, CT * DG], BF16, isOutput=False)
    Wo = nc.declare_dram_parameter("Wo", [P, CT * C], BF16, isOutput=False)
    bqs = nc.declare_dram_parameter("bqs", [P, CT], F32, isOutput=False)
    bks = nc.declare_dram_parameter("bks", [P, 2], F32, isOutput=False)
    bv = nc.declare_dram_parameter("bv", [DG], BF16, isOutput=False)
    bo = nc.declare_dram_parameter("bo", [C], BF16, isOutput=False)
    y = nc.declare_dram_parameter("y", [NQ, C], F32, isOutput=True)
    dbg = os.environ.get("KDBG") == "1"
    if dbg:
        qTo = nc.declare_dram_parameter("qTo", [P, CT, NQ], F32, isOutput=True)
        kTo = nc.declare_dram_parameter("kTo", [P, 2, N], F32, isOutput=True)
        vAo = nc.declare_dram_parameter("vAo", [P, KC, G, HD + 1], F32,
                                        isOutput=True)
        oTo = nc.declare_dram_parameter("oTo", [P, CT, NQ], F32, isOutput=True)
        dno = nc.declare_dram_parameter("dno", [P, 4, NQ], F32, isOutput=True)
        dnro = nc.declare_dram_parameter("dnro", [P, 4, NQ], F32, isOutput=True)

    with tile.TileContext(nc) as tc:
        with tc.tile_pool(name="main", bufs=1) as main:
            # -------- persistent tiles --------
            qT = main.tile([P, CT, NQ], BF16)          # Q^T  d-chunk x q
            kT = main.tile([P, 2, N], BF16)            # K^T  dg-chunk x k
            vA = main.tile([P, KC, G, HD + 1], BF16)   # [V | 1] per k-chunk
            oT = main.tile([P, CT, NQ], BF16)          # normalized O^T
            wo = main.tile([P, CT, C], BF16)
            dn = main.tile([P, 4, NQ], F32)            # denominators (spread)
            dnr = main.tile([P, 4, NQ], F32)           # their reciprocals
            e4 = main.tile([P, HD], BF16)
            dnb = main.tile([P, 4, NQ], BF16)               # norm-broadcast lhsT
            ypart = main.tile([P, CT, 512], F32)       # O-proj partials
            dumw = main.tile([P, NQ], BF16)            # HAM warm-up scratch
            bqk = main.tile([P, CT + 2], F32)          # bq (d-chunked) | bk
            ones1 = main.tile([1, P], BF16)
            bvr = main.tile([1, DG], BF16)
            bor = main.tile([1, C], BF16)
            bqp = bqk[:, 0:CT]
            bkp = bqk[:, CT:CT + 2]

            with tc.tile_pool(name="proj", bufs=1) as proj:
                xbTs = proj.tile([P, CT, N], BF16)
                wq = proj.tile([P, CT, C], BF16)
                wk = proj.tile([P, CT, DG], BF16)
                wv = proj.tile([P, CT, DG], BF16)

                # input DMAs, in consumption order (contiguous layouts)
                nc.sync.dma_start(
                    out=wk[:], in_=Wk.rearrange("p (t d) -> p t d", t=CT))
                for t in range(CT):
                    nc.sync.dma_start(out=xbTs[:, t, :],
                                      in_=xbT[t * P:(t + 1) * P, :])
                nc.sync.dma_start(
                    out=wv[:], in_=Wv.rearrange("p (t d) -> p t d", t=CT))
                # biases (contiguous, host-pre-shuffled)
                nc.sync.dma_start(out=bqp, in_=bqs[:, :])
                nc.sync.dma_start(out=bkp, in_=bks[:, :])
                nc.sync.dma_start(out=bvr[:],
                                  in_=bv.rearrange("(o d) -> o d", o=1))
                nc.sync.dma_start(out=bor[:],
                                  in_=bo.rearrange("(o d) -> o d", o=1))
                nc.sync.dma_start(out=ones1[:],
                                  in_=nc.inline_tensor(
                                      np.ones((1, P), NPBF), "ones1")[:])
                # e4: ones rows at partitions 0/32/64/96 (K=1 bcast lhsT)
                em = np.zeros((P, HD), NPBF)
                em[0::32, :] = 1.0
                nc.sync.dma_start(out=e4[:],
                                  in_=nc.inline_tensor(em, "e4")[:])
                # vA ones column via memset (GpSimd; V copies overwrite 0:HD)
                nc.gpsimd.memset(vA[:], 1.0)
                nc.gpsimd.memset(dumw[:], 0.0)
                nc.sync.dma_start(
                    out=wq[:], in_=Wq.rearrange("p (t d) -> p t d", t=CT))
                nc.sync.dma_start(
                    out=wo[:], in_=Wo.rearrange("p (t d) -> p t d", t=CT))

                with tc.tile_pool(name="pp", bufs=2, space="PSUM") as pp:
                    # ---- HAM warm-up: dummy matmuls on scratch data ----
                    # (no DMA deps -> PE busy from ~7us; un-throttles the
                    # clock gate before the real projections arrive)
                    for wu in range(40):
                        pwu = pp.tile([P, NQ], F32, tag="pk", name=f"wu{wu}")
                        nc.tensor.matmul(
                            pwu[:], dumw[:, 0:P], dumw[:],
                            start=True, stop=True)
                    # ---- K^T ----
                    for gt in range(2):
                        for nf in range(4):
                            pk = pp.tile([P, 512], F32, tag="pk")
                            for t in range(CT):
                                nc.tensor.matmul(
                                    pk[:], wk[:, t, gt * P:(gt + 1) * P],
                                    xbTs[:, t, nf * 512:(nf + 1) * 512],
                                    start=(t == 0), stop=(t == CT - 1))
                            nc.vector.tensor_scalar_add(
                                kT[:, gt, nf * 512:(nf + 1) * 512], pk[:],
                                bkp[:, gt:gt + 1])

                    # ---- V (natural layout) ----
                    for kc in range(KC):
                        pv = pp.tile([P, DG], F32, tag="pv")
                        for t in range(CT):
                            nc.tensor.matmul(
                                pv[:], xbTs[:, t, kc * P:(kc + 1) * P],
                                wv[:, t, :], start=(t == 0), stop=False)
                        nc.tensor.matmul(pv[:], ones1[:], bvr[:],
                                         start=False, stop=True)
                        nc.vector.tensor_copy(
                            vA[:, kc, :, 0:HD],
                            pv[:].rearrange("p (g d) -> p g d", g=G))

                    # ---- Q^T chunks 0..1 (rest interleaved into attention)
                    for dt_ in range(2):
                        pq = pp.tile([P, NQ], F32, tag="pk")
                        for t in range(CT):
                            nc.tensor.matmul(
                                pq[:], wq[:, t, dt_ * P:(dt_ + 1) * P],
                                xbTs[:, t, 0:NQ],
                                start=(t == 0), stop=(t == CT - 1))
                        nc.vector.tensor_scalar_add(
                            qT[:, dt_, :], pq[:], bqp[:, dt_:dt_ + 1])

                # -------- attention: 8 head pairs --------
                # PSUM banks: ps 2x2 + po 2x1 + qf 1 + pb 1 = 8
                with tc.tile_pool(name="ps", bufs=2, space="PSUM") as psp, \
                     tc.tile_pool(name="pt", bufs=3) as ptp, \
                     tc.tile_pool(name="po", bufs=1, space="PSUM") as pop, \
                     tc.tile_pool(name="qf", bufs=1, space="PSUM") as qfp, \
                     tc.tile_pool(name="pn", bufs=1, space="PSUM") as pnp:
                    opart_sched = {}
                    _oi = 0
                    for _pp in (6, 7):
                        for _w in (1, 5, 9, 13):
                            opart_sched[(_pp, _w)] = _oi
                            _oi += 1
                    for p_ in range(NPAIR):
                        g0, g1 = (2 * p_) % G, (2 * p_ + 1) % G
                        po0 = pop.tile([HD + 1, NQ], F32, tag="po0")
                        po1 = pop.tile([HD + 1, NQ], F32, tag="po1")
                        pos = (po0, po1)
                        dt_ = 2 + p_  # Q chunk computed as PE filler
                        pq_f = None
                        if dt_ < CT:
                            pq_f = qfp.tile([P, NQ], F32, tag="qf",
                                            name=f"pqf{p_}")
                        for kc in range(KC):
                            ps = psp.tile([P, 2, NQ], F32, tag="ps")
                            for hh in range(2):
                                r = hh * HD
                                g = g0 if hh == 0 else g1
                                nc.tensor.matmul(
                                    ps[:, hh, :],
                                    kT[r:r + HD, g // 2, kc * P:(kc + 1) * P],
                                    qT[r:r + HD, p_, :],
                                    start=True, stop=True)
                            pT = ptp.tile([P, 2, NQ], BF16, tag="pt")
                            nc.scalar.activation(
                                pT[:], ps[:],
                                mybir.ActivationFunctionType.Exp,
                                scale=SCALE)
                            for hh in range(2):
                                g = g0 if hh == 0 else g1
                                nc.tensor.matmul(
                                    pos[hh][:], vA[:, kc, g, :],
                                    pT[:, hh, :],
                                    start=(kc == 0), stop=(kc == KC - 1))
                            # PE filler: Q chunk spread over the pair
                            if pq_f is not None and kc % 2 == 0:
                                t = kc // 2
                                nc.tensor.matmul(
                                    pq_f[:], wq[:, t, dt_ * P:(dt_ + 1) * P],
                                    xbTs[:, t, 0:NQ],
                                    start=(t == 0), stop=(t == CT - 1))
                                if t == CT - 1:
                                    nc.vector.tensor_scalar_add(
                                        qT[:, dt_, :], pq_f[:],
                                        bqp[:, dt_:dt_ + 1])
                            # PE filler: O-proj partials (chunks 0..5)
                            oi = opart_sched.get((p_, kc))
                            if oi is not None:
                                m_, fh_ = oi // 2, oi % 2
                                nt = 6 if p_ == 6 else 7
                                pyp_ = qfp.tile([P, 512], F32, tag="qf",
                                                name=f"op{oi}")
                                for t in range(nt):
                                    nc.tensor.matmul(
                                        pyp_[:], oT[:, t, m_ * P:(m_ + 1) * P],
                                        wo[:, t, fh_ * 512:(fh_ + 1) * 512],
                                        start=(t == 0), stop=(t == nt - 1))
                                nc.vector.tensor_copy(ypart[:, oi, :], pyp_[:])

                        # ---- normalize pair p_ (per-pair, keeps PE warm) --
                        pi = 32 * ((2 * p_) % 4)   # 0 or 64
                        slot = p_ // 2
                        nc.vector.tensor_copy(
                            dn[pi:pi + 1, slot, :], po0[HD:HD + 1, :])
                        nc.vector.tensor_copy(
                            dn[pi + 32:pi + 33, slot, :], po1[HD:HD + 1, :])
                        nc.vector.tensor_copy(oT[0:HD, p_, :], po0[0:HD, :])
                        nc.vector.tensor_copy(oT[HD:P, p_, :], po1[0:HD, :])
                        # full-128-partition recip (base-64 slices are broken
                        # in the custom DVE op; re-reading the sibling pair's
                        # rows is idempotent, garbage rows are never read)
                        with nc.allow_low_precision(reason="recip"):
                            nc.vector.reciprocal_approx_fast(
                                dnr[:, slot, :], dn[:, slot, :])
                            nc.vector.tensor_copy(dnb[:, slot, :],
                                                  dnr[:, slot, :])
                        pb = pnp.tile([P, NQ], F32, tag="pb",
                                      name=f"pb{p_}")
                        nc.tensor.matmul(
                            pb[0:HD, :], e4[pi:pi + 1, :],
                            dnb[pi:pi + 1, slot, :],
                            start=True, stop=True,
                            tile_position=(pi, 0))
                        nc.tensor.matmul(
                            pb[HD:P, :], e4[pi + 32:pi + 33, :],
                            dnb[pi + 32:pi + 33, slot, :],
                            start=True, stop=True,
                            tile_position=(pi + 32, 64))
                        with nc.allow_low_precision(reason="bf16"):
                            nc.vector.tensor_mul(
                                oT[0:HD, p_, :], oT[0:HD, p_, :],
                                pb[0:HD, :])
                            nc.vector.tensor_mul(
                                oT[HD:P, p_, :], oT[HD:P, p_, :],
                                pb[HD:P, :])

            if dbg:
                with tc.tile_pool(name="dbg", bufs=1) as dbgp:
                    for (dst, srcT) in ((qTo, qT), (kTo, kT), (oTo, oT)):
                        f = dbgp.tile([P, 8 * NQ], F32, name=f"d{dst.name}")
                        nc.vector.tensor_copy(
                            f[:].rearrange("p (a b) -> p a b", a=srcT.shape[1]),
                            srcT[:])
                        nc.sync.dma_start(
                            out=dst.rearrange("p a b -> p (a b)"), in_=f[:])
                    fv = dbgp.tile([P, KC * G * (HD + 1)], F32, name="dv")
                    nc.vector.tensor_copy(
                        fv[:].rearrange("p (k g d) -> p k g d", k=KC, g=G),
                        vA[:])
                    nc.sync.dma_start(
                        out=vAo.rearrange("p k g d -> p (k g d)"), in_=fv[:])
                    nc.sync.dma_start(
                        out=dno.rearrange("p a b -> p (a b)"),
                        in_=dn[:].rearrange("p a b -> p (a b)"))
                    nc.sync.dma_start(
                        out=dnro.rearrange("p a b -> p (a b)"),
                        in_=dnr[:].rearrange("p a b -> p (a b)"))

            # -------- out-projection tail (chunks 6,7 + bias + partial) ---
            with tc.tile_pool(name="py", bufs=2, space="PSUM") as pyp, \
                 tc.tile_pool(name="ysb", bufs=2) as ysb:
                # keep PE warm across the final normalization latency
                for wu in range(18):
                    pwu = pyp.tile([P, 512], F32, tag="py", name=f"wt{wu}")
                    nc.tensor.matmul(
                        pwu[:], dumw[:, 0:P], dumw[:, 0:NQ],
                        start=True, stop=True)
                for m in range(NQ // P):
                    for fh in range(C // 512):
                        oi = m * 2 + fh
                        py = pyp.tile([P, 512], F32, tag="py")
                        ts_ = (6, 7) if oi < 4 else (7,)
                        for t in ts_:
                            nc.tensor.matmul(
                                py[:], oT[:, t, m * P:(m + 1) * P],
                                wo[:, t, fh * 512:(fh + 1) * 512],
                                start=(t == ts_[0]), stop=False)
                        nc.tensor.matmul(py[:], ones1[:],
                                         bor[0:1, fh * 512:(fh + 1) * 512],
                                         start=False, stop=True)
                        yt = ysb.tile([P, 512], F32, tag="yt")
                        nc.vector.tensor_add(yt[:], py[:], ypart[:, oi, :])
                        nc.sync.dma_start(
                            out=y[m * P:(m + 1) * P, fh * 512:(fh + 1) * 512],
                            in_=yt[:])

    nc.compile()
    return nc


def _get_nc():
    if "nc" not in _CACHE:
        _CACHE["nc"] = _build()
    return _CACHE["nc"]


LAST_RESULTS = None


def _shuf(w):
    # [C, D] -> [P, CT*D]: slot (p; t, d) = w[t*128 + p, d]
    w = np.asarray(w, np.float32).astype(NPBF)
    d = w.shape[1]
    return np.ascontiguousarray(
        w.reshape(CT, P, d).transpose(1, 0, 2).reshape(P, CT * d))


def kernel(x, Wq, bq, Wk, bk, Wv, bv, Wo, bo, trace=False, **trace_kwargs):
    x = np.asarray(x, dtype=np.float32)
    WqB = _shuf(Wq)
    WkB = _shuf(Wk)
    WvB = _shuf(Wv)
    WoB = _shuf(Wo)
    bqF = np.ascontiguousarray(
        np.asarray(bq, np.float32).reshape(CT, P).T)
    bkF = np.ascontiguousarray(
        np.asarray(bk, np.float32).reshape(2, P).T)
    bvB = np.ascontiguousarray(np.asarray(bv, np.float32)).astype(NPBF)
    boB = np.ascontiguousarray(np.asarray(bo, np.float32)).astype(NPBF)

    nc = _get_nc()
    in_maps = []
    for d in range(NCORES):
        b, j = d // SPLIT, d % SPLIT
        # Roll the key/token axis so this core's queries are columns 0:NQ.
        xbTr = np.ascontiguousarray(
            np.roll(x[b].T, -j * NQ, axis=1)).astype(NPBF)
        in_maps.append({
            "xbT": xbTr,
            "Wq": WqB, "Wk": WkB, "Wv": WvB, "Wo": WoB,
            "bqs": bqF, "bks": bkF, "bv": bvB, "bo": boB,
        })

    res = run_bass_kernel_spmd(nc, in_maps, core_ids=list(range(NCORES)),
                               trace=trace, **trace_kwargs)
    global LAST_RESULTS
    LAST_RESULTS = res

    out = np.empty((B, N, C), dtype=np.float32)
    for d in range(NCORES):
        b, j = d // SPLIT, d % SPLIT
        out[b, j * NQ:(j + 1) * NQ, :] = res.results[d]["y"]
    return out


# revision 18
# speedup vs baseline: 1.8263x; 1.0314x over previous
"""GroupedQueryAttention Trainium2 kernel (v2).

Problem shapes (hardcoded): x [2, 2048, 1024], H=16 heads, G=4 kv-groups,
head_dim=64.  out = softmax((xWq)(xWk)^T / 8) (xWv) Wo + biases.

Sharding: 8 cores, core d = (b, j) with b = d // 4, j = d % 4.
Each core computes the full attention output for batch b, query rows
[512j, 512j+512), all 16 heads.  The token axis of x^T is rolled per-core
so queries are always columns 0:512 (attention is permutation-invariant
over keys), keeping the SPMD program identical across cores.

v2 design (vs v1 baseline, 430us):
  - all matmul operands bf16 (host-cast); fp32 PSUM accumulation
  - score matmuls row-tiled: head pairs (2p, 2p+1) occupy PE array rows
    0:64 / 64:128 concurrently (head_dim=64 contraction)
  - scores land in PSUM as bf16 so an exp window of 2048 elem/partition
    fits in 2 banks; double-buffered -> ScalarE (the only exp engine,
    ~128us floor) runs back-to-back and paces the kernel
  - AV consumes exp'd probabilities [V|1]-augmented (denominator row 64)
  - per-pair denominators copied to spread partitions, batched
    reciprocal_approx_fast, broadcast back via a tiny K=2 matmul
  - Q-projection chunks interleaved into the attention phase as PE filler
"""

import os

import ml_dtypes
import numpy as np

import concourse.bacc as bacc
import concourse.mybir as mybir
import concourse.tile as tile
from concourse.bass_utils import run_bass_kernel_spmd

# ---- problem constants (hardcoded per contract) ----
B, N, C = 2, 2048, 1024
H, G, HD = 16, 4, 64
DG = G * HD            # 256
NCORES = 8
SPLIT = NCORES // B    # 4 query splits per batch
NQ = N // SPLIT        # 512 query rows per core
P = 128
CT = C // P            # 8 c-chunks
KC = N // P            # 16 k-chunks
NPAIR = H // 2         # 8 head pairs
NW = KC // 2           # 8 windows of 2 k-chunks per pair
SCALE = HD ** -0.5

F32 = mybir.dt.float32
BF16 = mybir.dt.bfloat16
NPBF = ml_dtypes.bfloat16

_CACHE = {}


def _build():
    nc = bacc.Bacc(None, target_bir_lowering=False)

    xbT = nc.declare_dram_parameter("xbT", [C, N], BF16, isOutput=False)
    # weights host-pre-shuffled to [p, t, d] layout for contiguous DMA
    Wq = nc.declare_dram_parameter("Wq", [P, CT * C], BF16, isOutput=False)
    Wk = nc.declare_dram_parameter("Wk", [P, CT * DG], BF16, isOutput=False)
    Wv = nc.declare_dram_parameter("Wv", [P, CT * DG], BF16, isOutput=False)
    Wo = nc.declare_dram_parameter("Wo", [P, CT * C], BF16, isOutput=False)
    bqs = nc.declare_dram_parameter("bqs", [P, CT], F32, isOutput=False)
    bks = nc.declare_dram_parameter("bks", [P, 2], F32, isOutput=False)
    bv = nc.declare_dram_parameter("bv", [DG], BF16, isOutput=False)
    bo = nc.declare_dram_parameter("bo", [C], BF16, isOutput=False)
    y = nc.declare_dram_parameter("y", [NQ, C], F32, isOutput=True)
    dbg = os.environ.get("KDBG") == "1"
    if dbg:
        qTo = nc.declare_dram_parameter("qTo", [P, CT, NQ], F32, isOutput=True)
        kTo = nc.declare_dram_parameter("kTo", [P, 2, N], F32, isOutput=True)
        vAo = nc.declare_dram_parameter("vAo", [P, KC, G, HD + 1], F32,
                                        isOutput=True)
        oTo = nc.declare_dram_parameter("oTo", [P, CT, NQ], F32, isOutput=True)
        dno = nc.declare_dram_parameter("dno", [P, 4, NQ], F32, isOutput=True)
        dnro = nc.declare_dram_parameter("dnro", [P, 4, NQ], F32, isOutput=True)

    with tile.TileContext(nc) as tc:
        with tc.tile_pool(name="main", bufs=1) as main:
            # -------- persistent tiles --------
            qT = main.tile([P, CT, NQ], BF16)          # Q^T  d-chunk x q
            kT = main.tile([P, 2, N], BF16)            # K^T  dg-chunk x k
            vA = main.tile([P, KC, G, HD + 1], BF16)   # [V | 1] per k-chunk
            oT = main.tile([P, CT, NQ], BF16)          # normalized O^T
            wo = main.tile([P, CT, C], BF16)
            dn = main.tile([P, 4, NQ], F32)            # denominators (spread)
            dnr = main.tile([P, 4, NQ], F32)           # their reciprocals
            e4 = main.tile([P, HD], BF16)
            dnb = main.tile([P, 4, NQ], BF16)               # norm-broadcast lhsT
            ypart = main.tile([P, CT, 512], F32)       # O-proj partials
            dumw = main.tile([P, NQ], BF16)            # HAM warm-up scratch
            bqk = main.tile([P, CT + 2], F32)          # bq (d-chunked) | bk
            ones1 = main.tile([1, P], BF16)
            bvr = main.tile([1, DG], BF16)
            bor = main.tile([1, C], BF16)
            bqp = bqk[:, 0:CT]
            bkp = bqk[:, CT:CT + 2]

            with tc.tile_pool(name="proj", bufs=1) as proj:
                xbTs = proj.tile([P, CT, N], BF16)
                wq = proj.tile([P, CT, C], BF16)
                wk = proj.tile([P, CT, DG], BF16)
                wv = proj.tile([P, CT, DG], BF16)

                # input DMAs, in consumption order (contiguous layouts)
                nc.sync.dma_start(
                    out=wk[:], in_=Wk.rearrange("p (t d) -> p t d", t=CT))
                for t in range(CT):
                    nc.sync.dma_start(out=xbTs[:, t, :],
                                      in_=xbT[t * P:(t + 1) * P, :])
                nc.sync.dma_start(
                    out=wv[:], in_=Wv.rearrange("p (t d) -> p t d", t=CT))
                # biases (contiguous, host-pre-shuffled)
                nc.sync.dma_start(out=bqp, in_=bqs[:, :])
                nc.sync.dma_start(out=bkp, in_=bks[:, :])
                nc.sync.dma_start(out=bvr[:],
                                  in_=bv.rearrange("(o d) -> o d", o=1))
                nc.sync.dma_start(out=bor[:],
                                  in_=bo.rearrange("(o d) -> o d", o=1))
                nc.sync.dma_start(out=ones1[:],
                                  in_=nc.inline_tensor(
                                      np.ones((1, P), NPBF), "ones1")[:])
                # e4: ones rows at partitions 0/32/64/96 (K=1 bcast lhsT)
                em = np.zeros((P, HD), NPBF)
                em[0::32, :] = 1.0
                nc.sync.dma_start(out=e4[:],
                                  in_=nc.inline_tensor(em, "e4")[:])
                # vA ones column via memset (GpSimd; V copies overwrite 0:HD)
                nc.gpsimd.memset(vA[:], 1.0)
                nc.gpsimd.memset(dumw[:], 0.0)
                nc.sync.dma_start(
                    out=wq[:], in_=Wq.rearrange("p (t d) -> p t d", t=CT))
                nc.sync.dma_start(
                    out=wo[:], in_=Wo.rearrange("p (t d) -> p t d", t=CT))

                with tc.tile_pool(name="pp", bufs=2, space="PSUM") as pp:
                    # ---- HAM warm-up: dummy matmuls on scratch data ----
                    # (no DMA deps -> PE busy from ~7us; un-throttles the
                    # clock gate before the real projections arrive)
                    for wu in range(40):
                        pwu = pp.tile([P, NQ], F32, tag="pk", name=f"wu{wu}")
                        nc.tensor.matmul(
                            pwu[:], dumw[:, 0:P], dumw[:],
                            start=True, stop=True)
                    # ---- K^T ----
                    for gt in range(2):
                        for nf in range(4):
                            pk = pp.tile([P, 512], F32, tag="pk")
                            for t in range(CT):
                                nc.tensor.matmul(
                                    pk[:], wk[:, t, gt * P:(gt + 1) * P],
                                    xbTs[:, t, nf * 512:(nf + 1) * 512],
                                    start=(t == 0), stop=(t == CT - 1))
                            nc.vector.tensor_scalar_add(
                                kT[:, gt, nf * 512:(nf + 1) * 512], pk[:],
                                bkp[:, gt:gt + 1])

                    # ---- V chunks 0..1 (rest JIT inside pair 0) ----
                    for kc in range(2):
                        pv = pp.tile([P, DG], F32, tag="pv")
                        for t in range(CT):
                            nc.tensor.matmul(
                                pv[:], xbTs[:, t, kc * P:(kc + 1) * P],
                                wv[:, t, :], start=(t == 0), stop=False)
                        nc.tensor.matmul(pv[:], ones1[:], bvr[:],
                                         start=False, stop=True)
                        nc.vector.tensor_copy(
                            vA[:, kc, :, 0:HD],
                            pv[:].rearrange("p (g d) -> p g d", g=G))

                    # ---- Q^T chunks 0,1,7 (2..6 interleaved into attention)
                    for dt_ in (0, 1, 7):
                        pq = pp.tile([P, NQ], F32, tag="pk")
                        for t in range(CT):
                            nc.tensor.matmul(
                                pq[:], wq[:, t, dt_ * P:(dt_ + 1) * P],
                                xbTs[:, t, 0:NQ],
                                start=(t == 0), stop=(t == CT - 1))
                        nc.vector.tensor_scalar_add(
                            qT[:, dt_, :], pq[:], bqp[:, dt_:dt_ + 1])

                # -------- attention: 8 head pairs --------
                # PSUM banks: ps 2x2 + po 2x1 + qf 1 + pb 1 = 8
                with tc.tile_pool(name="ps", bufs=2, space="PSUM") as psp, \
                     tc.tile_pool(name="pt", bufs=8) as ptp, \
                     tc.tile_pool(name="po", bufs=1, space="PSUM") as pop, \
                     tc.tile_pool(name="qf", bufs=1, space="PSUM") as qfp, \
                     tc.tile_pool(name="pn", bufs=1, space="PSUM") as pnp:
                    opart_sched = {}
                    _oi = 0
                    for _pp in (6, 7):
                        for _w in (1, 5, 9, 13):
                            opart_sched[(_pp, _w)] = _oi
                            _oi += 1
                    for p_ in range(NPAIR):
                        g0, g1 = (2 * p_) % G, (2 * p_ + 1) % G
                        po0 = pop.tile([HD + 1, NQ], F32, tag="po0")
                        po1 = pop.tile([HD + 1, NQ], F32, tag="po1")
                        pos = (po0, po1)
                        dt_ = 1 + p_  # Q chunk computed as PE filler
                        pq_f = None
                        if 1 <= p_ <= 5:
                            pq_f = qfp.tile([P, NQ], F32, tag="qf",
                                            name=f"pqf{p_}")
                        for kc in range(KC):
                            ps = psp.tile([P, 2, NQ], F32, tag="ps")
                            for hh in range(2):
                                r = hh * HD
                                g = g0 if hh == 0 else g1
                                nc.tensor.matmul(
                                    ps[:, hh, :],
                                    kT[r:r + HD, g // 2, kc * P:(kc + 1) * P],
                                    qT[r:r + HD, p_, :],
                                    start=True, stop=True)
                            pT = ptp.tile([P, 2, NQ], BF16, tag="pt")
                            nc.scalar.activation(
                                pT[:], ps[:],
                                mybir.ActivationFunctionType.Exp,
                                scale=SCALE)
                            for hh in range(2):
                                g = g0 if hh == 0 else g1
                                nc.tensor.matmul(
                                    pos[hh][:], vA[:, kc, g, :],
                                    pT[:, hh, :],
                                    start=(kc == 0), stop=(kc == KC - 1))
                            # PE filler: pair 0 computes V chunks JIT
                            if p_ == 0 and kc < KC - 2:
                                vkc = kc + 2
                                pvf = qfp.tile([P, 512], F32, tag="qf",
                                               name=f"pvf{vkc}")
                                for t in range(CT):
                                    nc.tensor.matmul(
                                        pvf[:, 0:DG],
                                        xbTs[:, t, vkc * P:(vkc + 1) * P],
                                        wv[:, t, :],
                                        start=(t == 0), stop=False)
                                nc.tensor.matmul(pvf[:, 0:DG], ones1[:],
                                                 bvr[:],
                                                 start=False, stop=True)
                                nc.vector.tensor_copy(
                                    vA[:, vkc, :, 0:HD],
                                    pvf[:, 0:DG].rearrange(
                                        "p (g d) -> p g d", g=G))
                            # PE filler: Q chunk spread over the pair
                            if pq_f is not None and kc % 2 == 0:
                                t = kc // 2
                                nc.tensor.matmul(
                                    pq_f[:], wq[:, t, dt_ * P:(dt_ + 1) * P],
                                    xbTs[:, t, 0:NQ],
                                    start=(t == 0), stop=(t == CT - 1))
                                if t == CT - 1:
                                    nc.vector.tensor_scalar_add(
                                        qT[:, dt_, :], pq_f[:],
                                        bqp[:, dt_:dt_ + 1])
                            # PE filler: O-proj partials (chunks 0..5)
                            oi = opart_sched.get((p_, kc))
                            if oi is not None:
                                m_, fh_ = oi // 2, oi % 2
                                nt = 6 if p_ == 6 else 7
                                pyp_ = qfp.tile([P, 512], F32, tag="qf",
                                                name=f"op{oi}")
                                for t in range(nt):
                                    nc.tensor.matmul(
                                        pyp_[:], oT[:, t, m_ * P:(m_ + 1) * P],
                                        wo[:, t, fh_ * 512:(fh_ + 1) * 512],
                                        start=(t == 0), stop=(t == nt - 1))
                                nc.vector.tensor_copy(ypart[:, oi, :], pyp_[:])

                        # ---- normalize pair p_ (per-pair, keeps PE warm) --
                        pi = 32 * ((2 * p_) % 4)   # 0 or 64
                        slot = p_ // 2
                        nc.vector.tensor_copy(
                            dn[pi:pi + 1, slot, :], po0[HD:HD + 1, :])
                        nc.vector.tensor_copy(
                            dn[pi + 32:pi + 33, slot, :], po1[HD:HD + 1, :])
                        nc.vector.tensor_copy(oT[0:HD, p_, :], po0[0:HD, :])
                        nc.vector.tensor_copy(oT[HD:P, p_, :], po1[0:HD, :])
                        # full-128-partition recip (base-64 slices are broken
                        # in the custom DVE op; re-reading the sibling pair's
                        # rows is idempotent, garbage rows are never read)
                        with nc.allow_low_precision(reason="recip"):
                            nc.vector.reciprocal_approx_fast(
                                dnr[:, slot, :], dn[:, slot, :])
                            nc.vector.tensor_copy(dnb[:, slot, :],
                                                  dnr[:, slot, :])
                        pb = pnp.tile([P, NQ], F32, tag="pb",
                                      name=f"pb{p_}")
                        nc.tensor.matmul(
                            pb[0:HD, :], e4[pi:pi + 1, :],
                            dnb[pi:pi + 1, slot, :],
                            start=True, stop=True,
                            tile_position=(pi, 0))
                        nc.tensor.matmul(
                            pb[HD:P, :], e4[pi + 32:pi + 33, :],
                            dnb[pi + 32:pi + 33, slot, :],
                            start=True, stop=True,
                            tile_position=(pi + 32, 64))
                        with nc.allow_low_precision(reason="bf16"):
                            nc.vector.tensor_mul(
                                oT[0:HD, p_, :], oT[0:HD, p_, :],
                                pb[0:HD, :])
                            nc.vector.tensor_mul(
                                oT[HD:P, p_, :], oT[HD:P, p_, :],
                                pb[HD:P, :])

            if dbg:
                with tc.tile_pool(name="dbg", bufs=1) as dbgp:
                    for (dst, srcT) in ((qTo, qT), (kTo, kT), (oTo, oT)):
                        f = dbgp.tile([P, 8 * NQ], F32, name=f"d{dst.name}")
                        nc.vector.tensor_copy(
                            f[:].rearrange("p (a b) -> p a b", a=srcT.shape[1]),
                            srcT[:])
                        nc.sync.dma_start(
                            out=dst.rearrange("p a b -> p (a b)"), in_=f[:])
                    fv = dbgp.tile([P, KC * G * (HD + 1)], F32, name="dv")
                    nc.vector.tensor_copy(
                        fv[:].rearrange("p (k g d) -> p k g d", k=KC, g=G),
                        vA[:])
                    nc.sync.dma_start(
                        out=vAo.rearrange("p k g d -> p (k g d)"), in_=fv[:])
                    nc.sync.dma_start(
                        out=dno.rearrange("p a b -> p (a b)"),
                        in_=dn[:].rearrange("p a b -> p (a b)"))
                    nc.sync.dma_start(
                        out=dnro.rearrange("p a b -> p (a b)"),
                        in_=dnr[:].rearrange("p a b -> p (a b)"))

            # -------- out-projection tail (chunks 6,7 + bias + partial) ---
            with tc.tile_pool(name="py", bufs=2, space="PSUM") as pyp, \
                 tc.tile_pool(name="ysb", bufs=2) as ysb:
                # keep PE warm across the final normalization latency
                for wu in range(18):
                    pwu = pyp.tile([P, 512], F32, tag="py", name=f"wt{wu}")
                    nc.tensor.matmul(
                        pwu[:], dumw[:, 0:P], dumw[:, 0:NQ],
                        start=True, stop=True)
                for m in range(NQ // P):
                    for fh in range(C // 512):
                        oi = m * 2 + fh
                        py = pyp.tile([P, 512], F32, tag="py")
                        ts_ = (6, 7) if oi < 4 else (7,)
                        for t in ts_:
                            nc.tensor.matmul(
                                py[:], oT[:, t, m * P:(m + 1) * P],
                                wo[:, t, fh * 512:(fh + 1) * 512],
                                start=(t == ts_[0]), stop=False)
                        nc.tensor.matmul(py[:], ones1[:],
                                         bor[0:1, fh * 512:(fh + 1) * 512],
                                         start=False, stop=True)
                        yt = ysb.tile([P, 512], F32, tag="yt")
                        nc.vector.tensor_add(yt[:], py[:], ypart[:, oi, :])
                        nc.sync.dma_start(
                            out=y[m * P:(m + 1) * P, fh * 512:(fh + 1) * 512],
                            in_=yt[:])

    nc.compile()
    return nc


def _get_nc():
    if "nc" not in _CACHE:
        _CACHE["nc"] = _build()
    return _CACHE["nc"]


LAST_RESULTS = None


def _shuf(w):
    # [C, D] -> [P, CT*D]: slot (p; t, d) = w[t*128 + p, d]
    w = np.asarray(w, np.float32).astype(NPBF)
    d = w.shape[1]
    return np.ascontiguousarray(
        w.reshape(CT, P, d).transpose(1, 0, 2).reshape(P, CT * d))


def kernel(x, Wq, bq, Wk, bk, Wv, bv, Wo, bo, trace=False, **trace_kwargs):
    x = np.asarray(x, dtype=np.float32)
    WqB = _shuf(Wq)
    WkB = _shuf(Wk)
    WvB = _shuf(Wv)
    WoB = _shuf(Wo)
    bqF = np.ascontiguousarray(
        np.asarray(bq, np.float32).reshape(CT, P).T)
    bkF = np.ascontiguousarray(
        np.asarray(bk, np.float32).reshape(2, P).T)
    bvB = np.ascontiguousarray(np.asarray(bv, np.float32)).astype(NPBF)
    boB = np.ascontiguousarray(np.asarray(bo, np.float32)).astype(NPBF)

    nc = _get_nc()
    in_maps = []
    for d in range(NCORES):
        b, j = d // SPLIT, d % SPLIT
        # Roll the key/token axis so this core's queries are columns 0:NQ.
        xbTr = np.ascontiguousarray(
            np.roll(x[b].T, -j * NQ, axis=1)).astype(NPBF)
        in_maps.append({
            "xbT": xbTr,
            "Wq": WqB, "Wk": WkB, "Wv": WvB, "Wo": WoB,
            "bqs": bqF, "bks": bkF, "bv": bvB, "bo": boB,
        })

    res = run_bass_kernel_spmd(nc, in_maps, core_ids=list(range(NCORES)),
                               trace=trace, **trace_kwargs)
    global LAST_RESULTS
    LAST_RESULTS = res

    out = np.empty((B, N, C), dtype=np.float32)
    for d in range(NCORES):
        b, j = d // SPLIT, d % SPLIT
        out[b, j * NQ:(j + 1) * NQ, :] = res.results[d]["y"]
    return out
